# revision 27
# baseline (speedup 1.0000x reference)
"""Trainium2 Bass kernel: entmax-1.5 along the last dim of x[8,16,1024,1024] f32.

Takes the FULL unsharded input, shards rows data-parallel across 8 NeuronCores
(pure rowwise op, no communication), runs a Bass/Tile kernel per core via
run_bass_kernel_spmd, and gathers the full output.

Per-row algorithm (d=1024, fp32). Solves for tau* with
f(t) = sum_j relu((x_j - t)/2)^2 - 1 = 0 (f convex decreasing), then
y = relu((x - tau*)/2)^2.

  1. seed: t0 from the exact support-8 closed form over the row's top-8
     values (DVE max8): t0 = m + (s1 - sqrt(s1^2 - 8*(s2-4)))/8 with
     m = row max, s1/s2 = sum / sum-of-squares of (top8 - m).
  2. two Newton steps t <- t + (A2 - 1)/(A1/2), each needing only two
     full-D passes:
       DVE  tensor_scalar(max,add-accum):  mo = max(x,t), A1 = sum mo - 1024t
                                           (= sum relu(x-t), benign noise)
       ACT  Square(0.5*mo - 0.5*t)+accum:  A2 = sum relu((x-t)/2)^2  (exact)
     Newton from below is monotone (t0 <= t1 <= t2 <= tau*), quadratic.
  3. fused final: y = Square(0.5*mo@t1 - 0.5*t2). Since t1 <= t2 <= tau*,
     relu(x-t2) == relu(max(x,t1)-t2) except for x in (t1, t2] where the
     error is <= ((t2-t1)/2)^2 ~ 1e-3 * 1e-3 -- far below tolerance.

Accuracy vs the sorted reference (measured on the real input distribution in
an fp32-faithful numpy sim): max rel err 2.4e-3, vs the 2e-2 gate.

Scheduling notes (verified against TimelineSim engine-busy accounting,
which tracked HW within ~5% for this kernel):
  - ACT is the busiest engine (2x Square-accum + ~2/3 of finals); every
    3rd tile's final y = ((mo-t2)/2)^2 runs on Pool (pool_final_mod=3).
  - All small [P,G] update ops run on DVE (small_eng='vector'): Pool's Q7
    sequencer costs ~1us dispatch per op vs DVE's ~45ns, and ~26 small
    ops/group made Pool dispatch a serial bottleneck.
  - Engine busy (model, per core): ACT ~405us, DMA ~373us, DVE ~328us,
    Pool ~175us; total 457us (HW measured ~480us via R-differencing).
"""

import sys

sys.path.insert(0, "/opt/trn_rl_repo")
sys.path.insert(0, "/opt/trn_rl_repo/concourse")

from contextlib import ExitStack

import numpy as np

D = 1024
P = 128
N_CORES = 8


def build_program(n_rows, group_tiles=8, dma_batch=2, debug=False,
                  xp_bufs=8, mos_bufs=4, yp_bufs=3, n_newton=2,
                  pool_final_mod=3, repeats=1, final_exact=False,
                  seed_ahead=False, smp_bufs=3, small_eng='vector',
                  newton_split=False, dve_s2_tiles=0, ablate=()):
    import concourse.bacc as bacc
    import concourse.tile as tile
    from concourse import mybir

    F32 = mybir.dt.float32
    ALU = mybir.AluOpType
    ACTF = mybir.ActivationFunctionType
    AX = mybir.AxisListType

    T = n_rows // P
    G = group_tiles
    assert n_rows % P == 0 and T % G == 0 and G % dma_batch == 0
    n_groups = T // G

    nc = bacc.Bacc(
        "TRN2", target_bir_lowering=False, debug=debug, enable_asserts=False
    )
    x = nc.dram_tensor("x", [n_rows, D], F32, kind="ExternalInput").ap()
    y = nc.dram_tensor("y", [n_rows, D], F32, kind="ExternalOutput").ap()

    with tile.TileContext(nc) as tc, ExitStack() as ctx:
        xp = ctx.enter_context(tc.tile_pool(name="xp", bufs=xp_bufs))
        mos = ctx.enter_context(tc.tile_pool(name="mos", bufs=mos_bufs))
        if not final_exact:
            mol = ctx.enter_context(tc.tile_pool(name="mol", bufs=2 * G + 2))
        atr = ctx.enter_context(tc.tile_pool(name="atr", bufs=3))
        zp = ctx.enter_context(tc.tile_pool(name="zp", bufs=2))
        yp = ctx.enter_context(tc.tile_pool(name="yp", bufs=yp_bufs))
        t8p = ctx.enter_context(tc.tile_pool(name="t8p", bufs=2))
        smp = ctx.enter_context(tc.tile_pool(name="smp", bufs=smp_bufs))

        # engine for small [P,G]-shaped ops: Pool's Q7 has a ~1us
        # per-op dispatch cost, DVE dispatches in ~45ns -- with ~26
        # small ops per group the Pool SEQ becomes a serial bottleneck.
        se = nc.vector if small_eng == 'vector' else nc.gpsimd

        state = {}

        def xv(g, j):
            return state[g]["xb"][j // dma_batch][:, j % dma_batch, :]

        def stage_load_seed(g):
            g_row0 = g * G * P
            xb = []
            for b in range(G // dma_batch):
                xt = xp.tile([P, dma_batch, D], F32, tag="x")
                r0 = g_row0 + b * dma_batch * P
                if "no_dma_in" not in ablate:
                    nc.sync.dma_start(
                        xt[:],
                        x[r0 : r0 + dma_batch * P, :].rearrange("(a p) m -> p a m", p=P),
                    )
                else:
                    nc.gpsimd.memset(xt[:, 0:1, 0:8], 0.5)
                xb.append(xt)
            state[g] = {"xb": xb}

            # top-8 per row (descending); m = t8[:,:,0]
            t8 = t8p.tile([P, G, 8], F32, tag="t8")
            for j in range(G):
                nc.vector.max(t8[:, j, :], xv(g, j))

            # seed from support-8 closed form (x units)
            u8 = smp.tile([P, G, 8], F32, tag="u8")
            se.tensor_sub(
                u8[:], t8[:], t8[:, :, 0:1].to_broadcast([P, G, 8])
            )
            sq8 = smp.tile([P, G, 8], F32, tag="sq8")
            se.tensor_mul(sq8[:], u8[:], u8[:])
            s1 = smp.tile([P, G], F32, tag="s1")
            nc.vector.tensor_reduce(s1[:], u8[:], axis=AX.X, op=ALU.add)
            s2 = smp.tile([P, G], F32, tag="s2")
            nc.vector.tensor_reduce(s2[:], sq8[:], axis=AX.X, op=ALU.add)
            q1 = smp.tile([P, G], F32, tag="q1")
            nc.scalar.activation(q1[:], s1[:], ACTF.Square)
            kd = smp.tile([P, G], F32, tag="kd")
            se.tensor_scalar(kd[:], s2[:], -4.0, 8.0, op0=ALU.add, op1=ALU.mult)
            di = smp.tile([P, G], F32, tag="di")
            se.tensor_sub(di[:], q1[:], kd[:])
            dn = smp.tile([P, G], F32, tag="dn")
            se.tensor_scalar_max(dn[:], di[:], 1e-30)
            root = smp.tile([P, G], F32, tag="root")
            nc.scalar.activation(root[:], dn[:], ACTF.Sqrt)
            num = smp.tile([P, G], F32, tag="num")
            se.tensor_sub(num[:], s1[:], root[:])
            th = smp.tile([P, G], F32, tag="th")
            se.tensor_scalar_mul(th[:], num[:], 0.125)
            t_cur = smp.tile([P, G], F32, tag="t0")
            se.tensor_add(t_cur[:], th[:], t8[:, :, 0])
            state[g]["t"] = t_cur

        def stage_newton_passes(g, it):
            t_cur = state[g]["t"]
            last = it == n_newton - 1
            A1 = smp.tile([P, G], F32, tag="A1")
            A2 = smp.tile([P, G], F32, tag="A2")
            tm = smp.tile([P, G], F32, tag="tm")
            se.tensor_scalar_mul(tm[:], t_cur[:], -1024.0)
            tb = smp.tile([P, G], F32, tag="tb")
            se.tensor_scalar_mul(tb[:], t_cur[:], -0.5)
            # First GS tiles of the FIRST iteration compute s2 on DVE
            # (tensor_tensor_reduce of mo^2, reconstructed to sum
            # relu((x-t)/2)^2 by [P,GS] smalls) to relieve ACT, which is
            # otherwise the busiest engine. N2 stays fully on ACT (exact).
            GS = dve_s2_tiles if it == 0 else 0
            Mt = None
            if GS:
                Mt = smp.tile([P, G], F32, tag="Mt")
            mo_tiles = []
            for j in range(G):
                t_col = t_cur[:, j : j + 1]
                mo = (mol if last and not final_exact else mos).tile(
                    [P, D], F32, tag="mo"
                )
                nc.vector.tensor_scalar(
                    mo[:], xv(g, j), t_col, tm[:, j : j + 1],
                    op0=ALU.max, op1=ALU.add,
                    accum_out=A1[:, j : j + 1],
                )
                sq = atr.tile([P, D], F32, tag="at")
                if j < GS:
                    nc.vector.tensor_tensor_reduce(
                        sq[:], mo[:], mo[:], 0.25, 0.0,
                        op0=ALU.mult, op1=ALU.add,
                        accum_out=Mt[:, j : j + 1],
                    )
                else:
                    nc.scalar.activation(
                        sq[:], mo[:], ACTF.Square, scale=0.5,
                        bias=tb[:, j : j + 1],
                        accum_out=A2[:, j : j + 1],
                    )
                mo_tiles.append(mo)
            if GS:
                # Mt[:, :GS] holds M = sum 0.25*mo^2; write the corrected
                # A2 = M - 0.5*t*A1 - 256*t^2 into A2's disjoint columns.
                sGS = slice(0, GS)
                t2s = smp.tile([P, G], F32, tag="t2s")
                se.tensor_mul(t2s[:, sGS], t_cur[:, sGS], t_cur[:, sGS])
                hh = smp.tile([P, G], F32, tag="hh")
                se.tensor_mul(hh[:, sGS], t_cur[:, sGS], A1[:, sGS])
                ww = smp.tile([P, G], F32, tag="ww")
                se.tensor_scalar_mul(ww[:, sGS], t2s[:, sGS], -256.0)
                uu = smp.tile([P, G], F32, tag="uu")
                nc.vector.scalar_tensor_tensor(
                    uu[:, sGS], hh[:, sGS], -0.5, ww[:, sGS],
                    op0=ALU.mult, op1=ALU.add,
                )
                nc.vector.scalar_tensor_tensor(
                    A2[:, sGS], Mt[:, sGS], 1.0, uu[:, sGS],
                    op0=ALU.mult, op1=ALU.add,
                )
            if last:
                state[g]["mo"] = mo_tiles
            state[g]["A"] = (A1, A2)

        def stage_newton_update(g, it):
            t_cur = state[g]["t"]
            A1, A2 = state[g]["A"]
            # t += (A2 - 1) / max(A1/2, eps)
            s1h = smp.tile([P, G], F32, tag="s1h")
            se.tensor_scalar(
                s1h[:], A1[:], 0.5, 1e-20, op0=ALU.mult, op1=ALU.max
            )
            rs = smp.tile([P, G], F32, tag="rs")
            nc.vector.reciprocal(rs[:], s1h[:])
            am = smp.tile([P, G], F32, tag="am")
            se.tensor_scalar(am[:], A2[:], -1.0, None, op0=ALU.add)
            dtv = smp.tile([P, G], F32, tag="dtv")
            se.tensor_mul(dtv[:], am[:], rs[:])
            t_new = smp.tile([P, G], F32, tag="tn")
            se.tensor_add(t_new[:], dtv[:], t_cur[:])
            state[g]["t"] = t_new

        def stage_newton(g, it):
            stage_newton_passes(g, it)
            stage_newton_update(g, it)

        def stage_final_exact(g):
            # y = Square(0.5*max(x,t2) - 0.5*t2): Pool computes
            # h = 0.5*max(x,t2) (2-op tensor_scalar, proven (AP,const)
            # slotting), ACT squares with bias. Exact final (no fused-mo
            # approximation); x stays alive until here, no mol pool.
            g_row0 = g * G * P
            t_cur = state[g]["t"]
            tb2 = smp.tile([P, G], F32, tag="tb2")
            se.tensor_scalar_mul(tb2[:], t_cur[:], -0.5)
            yt = None
            for j in range(G):
                if j % dma_batch == 0:
                    yt = yp.tile([P, dma_batch, D], F32, tag="y")
                h = mos.tile([P, D], F32, tag="h")
                nc.gpsimd.tensor_scalar(
                    h[:], xv(g, j), t_cur[:, j : j + 1], 0.5,
                    op0=ALU.max, op1=ALU.mult,
                )
                nc.scalar.activation(
                    yt[:, j % dma_batch, :], h[:], ACTF.Square, scale=1.0,
                    bias=tb2[:, j : j + 1],
                )
                if (j + 1) % dma_batch == 0:
                    r0 = g_row0 + (j + 1 - dma_batch) * P
                    if "no_dma_out" not in ablate:
                        nc.sync.dma_start(
                            y[r0 : r0 + dma_batch * P, :].rearrange(
                                "(a p) m -> p a m", p=P
                            ),
                            yt[:],
                        )

        def stage_final(g):
            if final_exact:
                return stage_final_exact(g)
            g_row0 = g * G * P
            t_cur = state[g]["t"]
            tb2 = smp.tile([P, G], F32, tag="tb2")
            se.tensor_scalar_mul(tb2[:], t_cur[:], -0.5)
            nt2 = smp.tile([P, G], F32, tag="nt2")
            se.tensor_scalar_mul(nt2[:], t_cur[:], -1.0)
            yt = None
            for j in range(G):
                if j % dma_batch == 0:
                    yt = yp.tile([P, dma_batch, D], F32, tag="y")
                mo = state[g]["mo"][j]
                if pool_final_mod and j % pool_final_mod == 0:
                    # Pool path: z = (mo + (-t2)) * 0.5; y = z*z (relieves
                    # ACT, which otherwise sits above the HBM roofline).
                    # Scalar operands follow the HW-proven (AP, const)
                    # slotting: scalar1 = per-partition AP, scalar2 = const.
                    zt = zp.tile([P, D], F32, tag="z")
                    nc.gpsimd.tensor_scalar(
                        zt[:], mo[:], nt2[:, j : j + 1], 0.5,
                        op0=ALU.add, op1=ALU.mult,
                    )
                    nc.gpsimd.tensor_mul(yt[:, j % dma_batch, :], zt[:], zt[:])
                else:
                    nc.scalar.activation(
                        yt[:, j % dma_batch, :], mo[:], ACTF.Square, scale=0.5,
                        bias=tb2[:, j : j + 1],
                    )
                if (j + 1) % dma_batch == 0:
                    r0 = g_row0 + (j + 1 - dma_batch) * P
                    if "no_dma_out" not in ablate:
                        nc.sync.dma_start(
                            y[r0 : r0 + dma_batch * P, :].rearrange(
                                "(a p) m -> p a m", p=P
                            ),
                            yt[:],
                        )
            state[g]["mo"] = None

        # pair-pipelined emission: two groups interleaved so each engine's
        # in-order stream has ready work while the other group's serial
        # t-update chain drains. With seed_ahead, the NEXT pair's load+seed
        # (whose 8x DVE max8 would otherwise starve ACT between pairs) is
        # emitted in the middle of the current pair's compute. repeats>1
        # re-emits the whole pass (same input, same output bytes) for
        # steady-state throughput measurement by R-differencing.
        def emit_pass():
            pairs = [
                [g for g in (p0, p0 + 1) if g < n_groups]
                for p0 in range(0, n_groups, 2)
            ]
            def emit_newton(pair, it):
                if newton_split:
                    # all big passes of the pair first, then both update
                    # chains -- the cross-engine A2 wait of group g's update
                    # is covered by g+1's passes on every engine.
                    for g in pair:
                        stage_newton_passes(g, it)
                    for g in pair:
                        stage_newton_update(g, it)
                else:
                    for g in pair:
                        stage_newton(g, it)

            if not seed_ahead:
                for pair in pairs:
                    for g in pair:
                        stage_load_seed(g)
                    for it in range(n_newton):
                        emit_newton(pair, it)
                    for g in pair:
                        stage_final(g)
                return
            for g in pairs[0]:
                stage_load_seed(g)
            for p, pair in enumerate(pairs):
                emit_newton(pair, 0)
                if p + 1 < len(pairs):
                    for g in pairs[p + 1]:
                        stage_load_seed(g)
                for it in range(1, n_newton):
                    emit_newton(pair, it)
                for g in pair:
                    stage_final(g)

        for _ in range(repeats):
            emit_pass()

    nc.compile()
    return nc


_PROGRAM = None
_PROGRAM_ROWS = None


def _get_program(rows_per_core):
    global _PROGRAM, _PROGRAM_ROWS
    if _PROGRAM is None or _PROGRAM_ROWS != rows_per_core:
        _PROGRAM = build_program(rows_per_core)
        _PROGRAM_ROWS = rows_per_core
    return _PROGRAM


def run_sharded(flat_x, trace=False):
    """flat_x: [n_rows, 1024] fp32. Returns (y, BassKernelResults)."""
    from concourse.bass_utils import run_bass_kernel_spmd

    n_rows = flat_x.shape[0]
    rows_per = n_rows // N_CORES
    assert rows_per * N_CORES == n_rows
    nc = _get_program(rows_per)
    in_maps = [
        {"x": np.ascontiguousarray(flat_x[i * rows_per : (i + 1) * rows_per])}
        for i in range(N_CORES)
    ]
    res = run_bass_kernel_spmd(nc, in_maps, list(range(N_CORES)), trace=trace)
    y = np.concatenate([res.results[i]["y"] for i in range(N_CORES)], axis=0)
    return y, res


def kernel(x):
    x = np.ascontiguousarray(np.asarray(x), dtype=np.float32)
    orig_shape = x.shape
    flat = x.reshape(-1, D)
    y, _ = run_sharded(flat)
    return y.reshape(orig_shape)


# revision 29
# speedup vs baseline: 1.1658x; 1.1658x over previous
"""Trainium2 Bass kernel: entmax-1.5 along the last dim of x[8,16,1024,1024] f32.

Takes the FULL unsharded input, shards rows data-parallel across 8 NeuronCores
(pure rowwise op, no communication), runs a Bass/Tile kernel per core via
run_bass_kernel_spmd, and gathers the full output.

Per-row algorithm (d=1024, fp32). Solves for tau* with
f(t) = sum_j relu((x_j - t)/2)^2 - 1 = 0 (f convex decreasing), then
y = relu((x - tau*)/2)^2.

  1. seed: t0 from the exact support-8 closed form over the row's top-8
     values (DVE max8): t0 = m + (s1 - sqrt(s1^2 - 8*(s2-4)))/8 with
     m = row max, s1/s2 = sum / sum-of-squares of (top8 - m).
  2. two Newton steps t <- t + (A2 - 1)/(A1/2), each needing only two
     full-D passes:
       DVE  tensor_scalar(max,add-accum):  mo = max(x,t), A1 = sum mo - 1024t
                                           (= sum relu(x-t), benign noise)
       ACT  Square(0.5*mo - 0.5*t)+accum:  A2 = sum relu((x-t)/2)^2  (exact)
     Newton from below is monotone (t0 <= t1 <= t2 <= tau*), quadratic.
  3. fused final: y = Square(0.5*mo@t1 - 0.5*t2). Since t1 <= t2 <= tau*,
     relu(x-t2) == relu(max(x,t1)-t2) except for x in (t1, t2] where the
     error is <= ((t2-t1)/2)^2 ~ 1e-3 * 1e-3 -- far below tolerance.

Accuracy vs the sorted reference (measured on the real input distribution in
an fp32-faithful numpy sim): max rel err 2.4e-3, vs the 2e-2 gate.

Scheduling notes:
  - ACT is the busiest engine (2x Square-accum + ~2/3 of finals); every
    3rd tile's final y = ((mo-t2)/2)^2 runs on Pool (pool_final_mod=3).
  - Small [P,G] update ops run on Pool (small_eng='gpsimd'). The cost
    model prefers them on DVE (457 vs 475 us) but HW disagrees (580 vs
    499 us measured via R-differencing) -- HW truth wins.
  - Engine busy (model, per core): ACT ~405us, DMA ~373us, DVE ~303us,
    Pool ~217us; HW steady-state ~499us via R-differencing.
"""

import sys

sys.path.insert(0, "/opt/trn_rl_repo")
sys.path.insert(0, "/opt/trn_rl_repo/concourse")

from contextlib import ExitStack

import numpy as np

D = 1024
P = 128
N_CORES = 8


def build_program(n_rows, group_tiles=8, dma_batch=2, debug=False,
                  xp_bufs=8, mos_bufs=4, yp_bufs=3, n_newton=2,
                  pool_final_mod=3, repeats=1, final_exact=False,
                  seed_ahead=False, smp_bufs=3, small_eng='gpsimd',
                  newton_split=False, dve_s2_tiles=0, ablate=()):
    import concourse.bacc as bacc
    import concourse.tile as tile
    from concourse import mybir

    F32 = mybir.dt.float32
    ALU = mybir.AluOpType
    ACTF = mybir.ActivationFunctionType
    AX = mybir.AxisListType

    T = n_rows // P
    G = group_tiles
    assert n_rows % P == 0 and T % G == 0 and G % dma_batch == 0
    n_groups = T // G

    nc = bacc.Bacc(
        "TRN2", target_bir_lowering=False, debug=debug, enable_asserts=False
    )
    x = nc.dram_tensor("x", [n_rows, D], F32, kind="ExternalInput").ap()
    y = nc.dram_tensor("y", [n_rows, D], F32, kind="ExternalOutput").ap()

    with tile.TileContext(nc) as tc, ExitStack() as ctx:
        xp = ctx.enter_context(tc.tile_pool(name="xp", bufs=xp_bufs))
        mos = ctx.enter_context(tc.tile_pool(name="mos", bufs=mos_bufs))
        if not final_exact:
            mol = ctx.enter_context(tc.tile_pool(name="mol", bufs=2 * G + 2))
        atr = ctx.enter_context(tc.tile_pool(name="atr", bufs=3))
        zp = ctx.enter_context(tc.tile_pool(name="zp", bufs=2))
        yp = ctx.enter_context(tc.tile_pool(name="yp", bufs=yp_bufs))
        t8p = ctx.enter_context(tc.tile_pool(name="t8p", bufs=2))
        smp = ctx.enter_context(tc.tile_pool(name="smp", bufs=smp_bufs))

        # engine for small [P,G]-shaped ops: Pool's Q7 has a ~1us
        # per-op dispatch cost, DVE dispatches in ~45ns -- with ~26
        # small ops per group the Pool SEQ becomes a serial bottleneck.
        se = nc.vector if small_eng == 'vector' else nc.gpsimd

        state = {}

        def xv(g, j):
            return state[g]["xb"][j // dma_batch][:, j % dma_batch, :]

        def stage_load_seed(g):
            g_row0 = g * G * P
            xb = []
            for b in range(G // dma_batch):
                xt = xp.tile([P, dma_batch, D], F32, tag="x")
                r0 = g_row0 + b * dma_batch * P
                if "no_dma_in" not in ablate:
                    nc.sync.dma_start(
                        xt[:],
                        x[r0 : r0 + dma_batch * P, :].rearrange("(a p) m -> p a m", p=P),
                    )
                else:
                    nc.gpsimd.memset(xt[:, 0:1, 0:8], 0.5)
                xb.append(xt)
            state[g] = {"xb": xb}

            # top-8 per row (descending); m = t8[:,:,0]
            t8 = t8p.tile([P, G, 8], F32, tag="t8")
            for j in range(G):
                nc.vector.max(t8[:, j, :], xv(g, j))

            # seed from support-8 closed form (x units)
            u8 = smp.tile([P, G, 8], F32, tag="u8")
            se.tensor_sub(
                u8[:], t8[:], t8[:, :, 0:1].to_broadcast([P, G, 8])
            )
            sq8 = smp.tile([P, G, 8], F32, tag="sq8")
            se.tensor_mul(sq8[:], u8[:], u8[:])
            s1 = smp.tile([P, G], F32, tag="s1")
            nc.vector.tensor_reduce(s1[:], u8[:], axis=AX.X, op=ALU.add)
            s2 = smp.tile([P, G], F32, tag="s2")
            nc.vector.tensor_reduce(s2[:], sq8[:], axis=AX.X, op=ALU.add)
            q1 = smp.tile([P, G], F32, tag="q1")
            nc.scalar.activation(q1[:], s1[:], ACTF.Square)
            kd = smp.tile([P, G], F32, tag="kd")
            se.tensor_scalar(kd[:], s2[:], -4.0, 8.0, op0=ALU.add, op1=ALU.mult)
            di = smp.tile([P, G], F32, tag="di")
            se.tensor_sub(di[:], q1[:], kd[:])
            dn = smp.tile([P, G], F32, tag="dn")
            se.tensor_scalar_max(dn[:], di[:], 1e-30)
            root = smp.tile([P, G], F32, tag="root")
            nc.scalar.activation(root[:], dn[:], ACTF.Sqrt)
            num = smp.tile([P, G], F32, tag="num")
            se.tensor_sub(num[:], s1[:], root[:])
            th = smp.tile([P, G], F32, tag="th")
            se.tensor_scalar_mul(th[:], num[:], 0.125)
            t_cur = smp.tile([P, G], F32, tag="t0")
            se.tensor_add(t_cur[:], th[:], t8[:, :, 0])
            state[g]["t"] = t_cur

        def stage_newton_passes(g, it):
            t_cur = state[g]["t"]
            last = it == n_newton - 1
            A1 = smp.tile([P, G], F32, tag="A1")
            A2 = smp.tile([P, G], F32, tag="A2")
            tm = smp.tile([P, G], F32, tag="tm")
            se.tensor_scalar_mul(tm[:], t_cur[:], -1024.0)
            tb = smp.tile([P, G], F32, tag="tb")
            se.tensor_scalar_mul(tb[:], t_cur[:], -0.5)
            # First GS tiles of the FIRST iteration compute s2 on DVE
            # (tensor_tensor_reduce of mo^2, reconstructed to sum
            # relu((x-t)/2)^2 by [P,GS] smalls) to relieve ACT, which is
            # otherwise the busiest engine. N2 stays fully on ACT (exact).
            GS = dve_s2_tiles if it == 0 else 0
            Mt = None
            if GS:
                Mt = smp.tile([P, G], F32, tag="Mt")
            mo_tiles = []
            for j in range(G):
                t_col = t_cur[:, j : j + 1]
                mo = (mol if last and not final_exact else mos).tile(
                    [P, D], F32, tag="mo"
                )
                nc.vector.tensor_scalar(
                    mo[:], xv(g, j), t_col, tm[:, j : j + 1],
                    op0=ALU.max, op1=ALU.add,
                    accum_out=A1[:, j : j + 1],
                )
                sq = atr.tile([P, D], F32, tag="at")
                if j < GS:
                    nc.vector.tensor_tensor_reduce(
                        sq[:], mo[:], mo[:], 0.25, 0.0,
                        op0=ALU.mult, op1=ALU.add,
                        accum_out=Mt[:, j : j + 1],
                    )
                else:
                    nc.scalar.activation(
                        sq[:], mo[:], ACTF.Square, scale=0.5,
                        bias=tb[:, j : j + 1],
                        accum_out=A2[:, j : j + 1],
                    )
                mo_tiles.append(mo)
            if GS:
                # Mt[:, :GS] holds M = sum 0.25*mo^2; write the corrected
                # A2 = M - 0.5*t*A1 - 256*t^2 into A2's disjoint columns.
                sGS = slice(0, GS)
                t2s = smp.tile([P, G], F32, tag="t2s")
                se.tensor_mul(t2s[:, sGS], t_cur[:, sGS], t_cur[:, sGS])
                hh = smp.tile([P, G], F32, tag="hh")
                se.tensor_mul(hh[:, sGS], t_cur[:, sGS], A1[:, sGS])
                ww = smp.tile([P, G], F32, tag="ww")
                se.tensor_scalar_mul(ww[:, sGS], t2s[:, sGS], -256.0)
                uu = smp.tile([P, G], F32, tag="uu")
                nc.vector.scalar_tensor_tensor(
                    uu[:, sGS], hh[:, sGS], -0.5, ww[:, sGS],
                    op0=ALU.mult, op1=ALU.add,
                )
                nc.vector.scalar_tensor_tensor(
                    A2[:, sGS], Mt[:, sGS], 1.0, uu[:, sGS],
                    op0=ALU.mult, op1=ALU.add,
                )
            if last:
                state[g]["mo"] = mo_tiles
            state[g]["A"] = (A1, A2)

        def stage_newton_update(g, it):
            t_cur = state[g]["t"]
            A1, A2 = state[g]["A"]
            # t += (A2 - 1) / max(A1/2, eps)
            s1h = smp.tile([P, G], F32, tag="s1h")
            se.tensor_scalar(
                s1h[:], A1[:], 0.5, 1e-20, op0=ALU.mult, op1=ALU.max
            )
            rs = smp.tile([P, G], F32, tag="rs")
            nc.vector.reciprocal(rs[:], s1h[:])
            am = smp.tile([P, G], F32, tag="am")
            se.tensor_scalar(am[:], A2[:], -1.0, None, op0=ALU.add)
            dtv = smp.tile([P, G], F32, tag="dtv")
            se.tensor_mul(dtv[:], am[:], rs[:])
            t_new = smp.tile([P, G], F32, tag="tn")
            se.tensor_add(t_new[:], dtv[:], t_cur[:])
            state[g]["t"] = t_new

        def stage_newton(g, it):
            stage_newton_passes(g, it)
            stage_newton_update(g, it)

        def stage_final_exact(g):
            # y = Square(0.5*max(x,t2) - 0.5*t2): Pool computes
            # h = 0.5*max(x,t2) (2-op tensor_scalar, proven (AP,const)
            # slotting), ACT squares with bias. Exact final (no fused-mo
            # approximation); x stays alive until here, no mol pool.
            g_row0 = g * G * P
            t_cur = state[g]["t"]
            tb2 = smp.tile([P, G], F32, tag="tb2")
            se.tensor_scalar_mul(tb2[:], t_cur[:], -0.5)
            yt = None
            for j in range(G):
                if j % dma_batch == 0:
                    yt = yp.tile([P, dma_batch, D], F32, tag="y")
                h = mos.tile([P, D], F32, tag="h")
                nc.gpsimd.tensor_scalar(
                    h[:], xv(g, j), t_cur[:, j : j + 1], 0.5,
                    op0=ALU.max, op1=ALU.mult,
                )
                nc.scalar.activation(
                    yt[:, j % dma_batch, :], h[:], ACTF.Square, scale=1.0,
                    bias=tb2[:, j : j + 1],
                )
                if (j + 1) % dma_batch == 0:
                    r0 = g_row0 + (j + 1 - dma_batch) * P
                    if "no_dma_out" not in ablate:
                        nc.sync.dma_start(
                            y[r0 : r0 + dma_batch * P, :].rearrange(
                                "(a p) m -> p a m", p=P
                            ),
                            yt[:],
                        )

        def stage_final(g):
            if final_exact:
                return stage_final_exact(g)
            g_row0 = g * G * P
            t_cur = state[g]["t"]
            tb2 = smp.tile([P, G], F32, tag="tb2")
            se.tensor_scalar_mul(tb2[:], t_cur[:], -0.5)
            nt2 = smp.tile([P, G], F32, tag="nt2")
            se.tensor_scalar_mul(nt2[:], t_cur[:], -1.0)
            yt = None
            for j in range(G):
                if j % dma_batch == 0:
                    yt = yp.tile([P, dma_batch, D], F32, tag="y")
                mo = state[g]["mo"][j]
                if pool_final_mod and j % pool_final_mod == 0:
                    # Pool path: z = (mo + (-t2)) * 0.5; y = z*z (relieves
                    # ACT, which otherwise sits above the HBM roofline).
                    # Scalar operands follow the HW-proven (AP, const)
                    # slotting: scalar1 = per-partition AP, scalar2 = const.
                    zt = zp.tile([P, D], F32, tag="z")
                    nc.gpsimd.tensor_scalar(
                        zt[:], mo[:], nt2[:, j : j + 1], 0.5,
                        op0=ALU.add, op1=ALU.mult,
                    )
                    nc.gpsimd.tensor_mul(yt[:, j % dma_batch, :], zt[:], zt[:])
                else:
                    nc.scalar.activation(
                        yt[:, j % dma_batch, :], mo[:], ACTF.Square, scale=0.5,
                        bias=tb2[:, j : j + 1],
                    )
                if (j + 1) % dma_batch == 0:
                    r0 = g_row0 + (j + 1 - dma_batch) * P
                    if "no_dma_out" not in ablate:
                        nc.sync.dma_start(
                            y[r0 : r0 + dma_batch * P, :].rearrange(
                                "(a p) m -> p a m", p=P
                            ),
                            yt[:],
                        )
            state[g]["mo"] = None

        # pair-pipelined emission: two groups interleaved so each engine's
        # in-order stream has ready work while the other group's serial
        # t-update chain drains. With seed_ahead, the NEXT pair's load+seed
        # (whose 8x DVE max8 would otherwise starve ACT between pairs) is
        # emitted in the middle of the current pair's compute. repeats>1
        # re-emits the whole pass (same input, same output bytes) for
        # steady-state throughput measurement by R-differencing.
        def emit_pass():
            pairs = [
                [g for g in (p0, p0 + 1) if g < n_groups]
                for p0 in range(0, n_groups, 2)
            ]
            def emit_newton(pair, it):
                if newton_split:
                    # all big passes of the pair first, then both update
                    # chains -- the cross-engine A2 wait of group g's update
                    # is covered by g+1's passes on every engine.
                    for g in pair:
                        stage_newton_passes(g, it)
                    for g in pair:
                        stage_newton_update(g, it)
                else:
                    for g in pair:
                        stage_newton(g, it)

            if not seed_ahead:
                for pair in pairs:
                    for g in pair:
                        stage_load_seed(g)
                    for it in range(n_newton):
                        emit_newton(pair, it)
                    for g in pair:
                        stage_final(g)
                return
            for g in pairs[0]:
                stage_load_seed(g)
            for p, pair in enumerate(pairs):
                emit_newton(pair, 0)
                if p + 1 < len(pairs):
                    for g in pairs[p + 1]:
                        stage_load_seed(g)
                for it in range(1, n_newton):
                    emit_newton(pair, it)
                for g in pair:
                    stage_final(g)

        for _ in range(repeats):
            emit_pass()

    nc.compile()
    return nc


_PROGRAM = None
_PROGRAM_ROWS = None


def _get_program(rows_per_core):
    global _PROGRAM, _PROGRAM_ROWS
    if _PROGRAM is None or _PROGRAM_ROWS != rows_per_core:
        _PROGRAM = build_program(rows_per_core)
        _PROGRAM_ROWS = rows_per_core
    return _PROGRAM


def run_sharded(flat_x, trace=False):
    """flat_x: [n_rows, 1024] fp32. Returns (y, BassKernelResults)."""
    from concourse.bass_utils import run_bass_kernel_spmd

    n_rows = flat_x.shape[0]
    rows_per = n_rows // N_CORES
    assert rows_per * N_CORES == n_rows
    nc = _get_program(rows_per)
    in_maps = [
        {"x": np.ascontiguousarray(flat_x[i * rows_per : (i + 1) * rows_per])}
        for i in range(N_CORES)
    ]
    res = run_bass_kernel_spmd(nc, in_maps, list(range(N_CORES)), trace=trace)
    y = np.concatenate([res.results[i]["y"] for i in range(N_CORES)], axis=0)
    return y, res


def kernel(x):
    x = np.ascontiguousarray(np.asarray(x), dtype=np.float32)
    orig_shape = x.shape
    flat = x.reshape(-1, D)
    y, _ = run_sharded(flat)
    return y.reshape(orig_shape)
